# revision 43
# baseline (speedup 1.0000x reference)
"""GRU-style GNN message-passing kernel for Trainium2 (8 NeuronCores, SPMD).

Reference computation (per node b, features 256, 8 neighbors):
    xr = x @ Wir.T + bir
    hr_n = hs_n @ Whr.T + bhr
    r_n = sigmoid(xr + hr_n)
    z = sigmoid(x @ Wiz.T + biz + h_sum @ Whz.T + bhz)
    s = sum_n r_n * hs_n
    n = tanh(x @ Win.T + bin + s @ Whn.T + bhn)
    out = (1 - z) * n + z * h_sum

Strategy: data-parallel over B=32768 across 8 cores (4096 rows each),
8 batch-chunks of 512 per core, feature-major on-chip layout
([256 features = 2 partition chunks of 128, batch free dim]).

The schedule is built around keeping the PE matmul stream dense (any PE
idle gap re-engages the HAM clock throttle and halves the PE clock):

  - Host-side pre-chunked HBM layouts: every DMA is a plain 2D copy
    with 2-16KB contiguous runs; one hs DMA per chunk (split per-pair
    for chunk 0 so pair 0 lands early); x and bf16-h_sum packed in one
    tensor; all weights in one need-ordered pack, DMA'd in two pieces.
  - Per chunk PE stream: xr matmuls, r-matmul pairs 0-1, the deferred
    [n-gate of chunk c-1 | z-gate of chunk c] block, pairs 2-3.  The
    DVE product tree of chunk c completes in chunk c+1, so PE never
    waits on it; z rides mid-chunk so ACT's start-of-chunk load is
    just the two xr bias-adds and ACT stays ahead of PE.
  - Each neighbor pair accumulates in one [128,2048] PSUM tile (4
    interleaved 512-wide regions: Whr k0/k1 matmuls + an identity
    matmul that adds the shared xr) and drains with a single wide
    sigmoid ACTIVATE.
  - DVE runs everything alias-free in the packed bf16 2x mode:
    products per pair, the add tree, and the final combine
    out = n + z*(h-n) in fp32 (h_sum kept fp32 for the dominant term).
"""

import sys
import numpy as np
from contextlib import ExitStack

sys.path.insert(0, "/opt/trn_rl_repo")

import ml_dtypes
import concourse.bacc as bacc
import concourse.tile as tile
from concourse import mybir
from concourse.bass_utils import run_bass_kernel_spmd

F32 = mybir.dt.float32
BF16 = mybir.dt.bfloat16
BF_NP = ml_dtypes.bfloat16

N_NEIGH, B, IN, H = 8, 32768, 256, 256
M = 8                    # cores
BL = B // M              # rows per core (4096)
NCH = 8                  # batch chunks per core
CW = BL // NCH           # chunk width (512)
NPAIR = N_NEIGH // 2     # neighbor pairs (4)

_cached = None  # compiled program, reused across kernel() calls

SIG = mybir.ActivationFunctionType.Sigmoid
TANH = mybir.ActivationFunctionType.Tanh

# weight pack column offsets (need-ordered: xr gate, z gate, r pairs, n)
W_OFF = {"wir": 0, "wiz": 512, "whz": 1024, "whr": 1536, "win": 2176,
         "whn": 2688}
ID_OFF = 2048
WP_COLS = 3200
WP_SPLIT = 1536  # piece A: wir/wiz/whz; piece B: whr/id/win/whn


def _build():
    nc = bacc.Bacc("TRN2", target_bir_lowering=False, debug=False, num_devices=M)

    # xbL packs x (cols 0:1024) and h_sum-bf16 (cols 1024:2048) per chunk
    xbL = nc.dram_tensor("xbL", [NCH, 128, 2048], BF16, kind="ExternalInput").ap()
    hfL = nc.dram_tensor("hfL", [NCH, 128, 1024], F32, kind="ExternalInput").ap()
    hsL = nc.dram_tensor("hsL", [NCH, 128, 8192], BF16,
                         kind="ExternalInput").ap()
    wpL = nc.dram_tensor("wpL", [128, WP_COLS], BF16, kind="ExternalInput").ap()
    # bias pack: col f*3+j holds feature-chunk f of (b_r, b_z, b_n)[j]
    biasp = nc.dram_tensor("biasp", [128, 6], F32, kind="ExternalInput").ap()
    outL = nc.dram_tensor("outL", [NCH, 128, 1024], F32, kind="ExternalOutput").ap()

    with tile.TileContext(nc) as tc, ExitStack() as ctx:
        cpool = ctx.enter_context(tc.tile_pool(name="const", bufs=1))
        x_pool = ctx.enter_context(tc.tile_pool(name="x", bufs=3))
        hf_pool = ctx.enter_context(tc.tile_pool(name="hf", bufs=3))
        hs_pool = ctx.enter_context(tc.tile_pool(name="hs", bufs=3))
        xr_pool = ctx.enter_context(tc.tile_pool(name="xr", bufs=2))
        z_pool = ctx.enter_context(tc.tile_pool(name="z", bufs=2))
        rc_pool = ctx.enter_context(tc.tile_pool(name="rc", bufs=2))
        pd_pool = ctx.enter_context(tc.tile_pool(name="pd", bufs=2))
        s_pool = ctx.enter_context(tc.tile_pool(name="s", bufs=2))
        n_pool = ctx.enter_context(tc.tile_pool(name="n", bufs=2))
        d_pool = ctx.enter_context(tc.tile_pool(name="d", bufs=2))
        o_pool = ctx.enter_context(tc.tile_pool(name="o", bufs=2))
        pp_pool = ctx.enter_context(tc.tile_pool(name="pp", bufs=2, space="PSUM"))

        # --- constants: weight pack in two need-ordered DMAs + biases ---
        wp_t = cpool.tile([128, WP_COLS], BF16, tag="wp", name="wp_t")
        nc.sync.dma_start(out=wp_t[:, 0:WP_SPLIT], in_=wpL[:, 0:WP_SPLIT])
        bias_t = cpool.tile([128, 6], F32, tag="biasp", name="bias_t")
        nc.sync.dma_start(out=bias_t[:, :], in_=biasp[:, :])
        # piece B (whr/id/win/whn) is DMA'd inside chunk 0, after the data
        # the very first matmuls need, so PE starts ~6us earlier

        def wcol(w, k, f):  # stationary [128,128] for weight w, k-chunk, f-chunk
            off = W_OFF[w] + k * 256 + f * 128
            return wp_t[:, off:off + 128]

        id_t = wp_t[:, ID_OFF:ID_OFF + 128]

        state = {}  # chunk -> tiles needed by the deferred n-gate/combine

        def emit_mid(c):
            """Between pairs 1 and 2 of chunk c: the deferred n-gate of
            chunk c-1, then its combine on DVE + store."""
            st = state.pop(c - 1)
            pn = pp_pool.tile([128, 2048], F32, tag="pp", name=f"pn_{c - 1}")
            for fi in range(2):
                o = pn[:, fi * 512:(fi + 1) * 512]
                nc.tensor.matmul(o, wcol("win", 0, fi), st["x"][:, 0:512],
                                 start=True, stop=False)
                nc.tensor.matmul(o, wcol("win", 1, fi), st["x"][:, 512:1024],
                                 start=False, stop=False)
                nc.tensor.matmul(o, wcol("whn", 0, fi), st["s"][:, 0:512],
                                 start=False, stop=False)
                nc.tensor.matmul(o, wcol("whn", 1, fi), st["s"][:, 512:1024],
                                 start=False, stop=True)
            nt = n_pool.tile([128, 1024], F32, tag="n", name=f"n_{c - 1}")
            for fi in range(2):
                nc.scalar.activation(nt[:, fi * 512:(fi + 1) * 512],
                                     pn[:, fi * 512:(fi + 1) * 512], TANH,
                                     bias=bias_t[:, fi * 3 + 2:fi * 3 + 3])
            # out = n + z * (h - n) on DVE, then store
            dt_ = d_pool.tile([128, 1024], F32, tag="d", name=f"d_{c - 1}")
            nc.vector.tensor_sub(dt_[:, :], st["hf"][:, :], nt[:, :])
            nc.vector.tensor_mul(dt_[:, :], st["z"][:, :], dt_[:, :])
            ot = o_pool.tile([128, 1024], F32, tag="o", name=f"o_{c - 1}")
            nc.vector.tensor_add(ot[:, :], nt[:, :], dt_[:, :])
            nc.sync.dma_start(out=outL[c - 1], in_=ot[:, :])

        def emit_last_tail(cc):
            """n-gate + combine for the final chunk, f-split to shorten the
            end-of-kernel serial chain."""
            st = state.pop(cc)
            pn = pp_pool.tile([128, 2048], F32, tag="pp", name=f"pn_{cc}")
            nt = n_pool.tile([128, 1024], F32, tag="n", name=f"n_{cc}")
            dt_ = d_pool.tile([128, 1024], F32, tag="d", name=f"d_{cc}")
            ot = o_pool.tile([128, 1024], F32, tag="o", name=f"o_{cc}")
            for fi in range(2):
                o = pn[:, fi * 512:(fi + 1) * 512]
                nc.tensor.matmul(o, wcol("win", 0, fi), st["x"][:, 0:512],
                                 start=True, stop=False)
                nc.tensor.matmul(o, wcol("win", 1, fi), st["x"][:, 512:1024],
                                 start=False, stop=False)
                nc.tensor.matmul(o, wcol("whn", 0, fi), st["s"][:, 0:512],
                                 start=False, stop=False)
                nc.tensor.matmul(o, wcol("whn", 1, fi), st["s"][:, 512:1024],
                                 start=False, stop=True)
                nc.scalar.activation(nt[:, fi * 512:(fi + 1) * 512],
                                     pn[:, fi * 512:(fi + 1) * 512], TANH,
                                     bias=bias_t[:, fi * 3 + 2:fi * 3 + 3])
                s_ = slice(fi * 512, (fi + 1) * 512)
                nc.vector.tensor_sub(dt_[:, s_], st["hf"][:, s_], nt[:, s_])
                nc.vector.tensor_mul(dt_[:, s_], st["z"][:, s_], dt_[:, s_])
                nc.vector.tensor_add(ot[:, s_], nt[:, s_], dt_[:, s_])
                nc.sync.dma_start(out=outL[cc][:, s_], in_=ot[:, s_])

        def emit_pair(c, p, hsc, xrt, rct, pdt, act_split=False):
            base = p * 2048
            # one PSUM tile per pair, (f, j, b) layout; four interleaved
            # 512-wide accumulation regions. whr matmuls first, the xr
            # identity adds last so xr is never waited on.
            pra = pp_pool.tile([128, 2048], F32, tag="pp", name=f"pr{p}_{c}")
            for fi in range(2):
                oj0 = pra[:, fi * 1024:fi * 1024 + 512]
                oj1 = pra[:, fi * 1024 + 512:fi * 1024 + 1024]
                nc.tensor.matmul(oj0, wcol("whr", 0, fi),
                                 hsc[:, base:base + 512],
                                 start=True, stop=False)
                nc.tensor.matmul(oj1, wcol("whr", 0, fi),
                                 hsc[:, base + 512:base + 1024],
                                 start=True, stop=False)
                nc.tensor.matmul(oj0, wcol("whr", 1, fi),
                                 hsc[:, base + 1024:base + 1536],
                                 start=False, stop=False)
                nc.tensor.matmul(oj1, wcol("whr", 1, fi),
                                 hsc[:, base + 1536:base + 2048],
                                 start=False, stop=False)
            for fi in range(2):
                nc.tensor.matmul(pra[:, fi * 1024:fi * 1024 + 512], id_t,
                                 xrt[:, fi * 512:(fi + 1) * 512],
                                 start=False, stop=True)
                nc.tensor.matmul(pra[:, fi * 1024 + 512:fi * 1024 + 1024],
                                 id_t, xrt[:, fi * 512:(fi + 1) * 512],
                                 start=False, stop=True)
            if act_split:
                # f-split activation/product path (used for the final pair
                # of the final chunk to shorten the tail chain)
                for fi in range(2):
                    hb = base + fi * 1024
                    nc.scalar.activation(rct[:, hb:hb + 1024],
                                         pra[:, fi * 1024:(fi + 1) * 1024], SIG)
                    nc.vector.tensor_mul(pdt[:, hb:hb + 1024],
                                         rct[:, hb:hb + 1024],
                                         hsc[:, hb:hb + 1024])
                    with nc.allow_low_precision(reason="bf16 neighbor sums"):
                        nc.vector.tensor_add(
                            rct[:, p * 1024 + fi * 512:p * 1024 + fi * 512 + 512],
                            pdt[:, hb:hb + 512],
                            pdt[:, hb + 512:hb + 1024])
            else:
                # r for pair p, both f chunks in one activation
                nc.scalar.activation(rct[:, base:base + 2048], pra[:, :], SIG)
                # products r*hs for the whole pair block (alias-free so the
                # DVE packed bf16 mode stays eligible)
                blk = slice(base, base + 2048)
                nc.vector.tensor_mul(pdt[:, blk], rct[:, blk], hsc[:, blk])
                # tree level 1: j0 + j1 per f chunk -> rc cols [p*1024, +1024)
                with nc.allow_low_precision(reason="bf16 neighbor sums"):
                    for fi in range(2):
                        nc.vector.tensor_add(
                            rct[:, p * 1024 + fi * 512:p * 1024 + fi * 512 + 512],
                            pdt[:, base + fi * 1024:base + fi * 1024 + 512],
                            pdt[:, base + fi * 1024 + 512:base + fi * 1024 + 1024])

        pend = {}  # chunk -> (rct, pdt) awaiting tree levels 2+3

        def emit_l23(cc, skip_l2a=False):
            rct, pdt = pend.pop(cc)
            sct = s_pool.tile([128, 1024], BF16, tag="s", name=f"s_{cc}")
            with nc.allow_low_precision(reason="bf16 neighbor sums"):
                if not skip_l2a:
                    nc.vector.tensor_add(pdt[:, 0:1024], rct[:, 0:1024],
                                         rct[:, 1024:2048])
                nc.vector.tensor_add(pdt[:, 1024:2048], rct[:, 2048:3072],
                                     rct[:, 3072:4096])
                nc.vector.tensor_add(sct[:, :], pdt[:, 0:1024],
                                     pdt[:, 1024:2048])
            state[cc]["s"] = sct

        for c in range(NCH):
            # --- input DMAs (plain 2D copies, 4-16KB contiguous runs);
            #     chunk 0's hs comes in per-pair so pair0 lands early ---
            xbt = x_pool.tile([128, 2048], BF16, tag="x", name=f"x_{c}")
            nc.sync.dma_start(out=xbt[:, :], in_=xbL[c])
            hsc = hs_pool.tile([128, 8192], BF16, tag="hs", name=f"hs_{c}")
            if c == 0:
                nc.sync.dma_start(out=hsc[:, 0:2048], in_=hsL[c][:, 0:2048])
                nc.sync.dma_start(out=wp_t[:, WP_SPLIT:WP_COLS],
                                  in_=wpL[:, WP_SPLIT:WP_COLS])
                for p in range(1, NPAIR):
                    nc.sync.dma_start(out=hsc[:, p * 2048:(p + 1) * 2048],
                                      in_=hsL[c][:, p * 2048:(p + 1) * 2048])
            else:
                nc.sync.dma_start(out=hsc[:, :], in_=hsL[c])
            hft = hf_pool.tile([128, 1024], F32, tag="hf", name=f"hf_{c}")
            nc.sync.dma_start(out=hft[:, :], in_=hfL[c])

            # tree tail of the previous chunk opens the DVE stream here,
            # filling what would otherwise be a DVE idle (re-throttle) gap
            if c > 0:
                emit_l23(c - 1)

            # --- chunk-front gates in one PSUM tile: xr = Wir@x + b_r
            #     (cols 0:1024) and the z pre-act (cols 1024:2048); this
            #     12-matmul block is the PE runway that covers ACT's
            #     end-of-previous-chunk lag ---
            pg = pp_pool.tile([128, 2048], F32, tag="pp", name=f"pg_{c}")
            for fi in range(2):
                o = pg[:, fi * 512:(fi + 1) * 512]
                nc.tensor.matmul(o, wcol("wir", 0, fi), xbt[:, 0:512],
                                 start=True, stop=False)
                nc.tensor.matmul(o, wcol("wir", 1, fi), xbt[:, 512:1024],
                                 start=False, stop=True)
            for fi in range(2):
                o = pg[:, 1024 + fi * 512:1024 + (fi + 1) * 512]
                nc.tensor.matmul(o, wcol("wiz", 0, fi), xbt[:, 0:512],
                                 start=True, stop=False)
                nc.tensor.matmul(o, wcol("wiz", 1, fi), xbt[:, 512:1024],
                                 start=False, stop=False)
                nc.tensor.matmul(o, wcol("whz", 0, fi), xbt[:, 1024:1536],
                                 start=False, stop=False)
                nc.tensor.matmul(o, wcol("whz", 1, fi), xbt[:, 1536:2048],
                                 start=False, stop=True)
            xrt = xr_pool.tile([128, 1024], BF16, tag="xr", name=f"xr_{c}")
            for fi in range(2):
                nc.scalar.add(xrt[:, fi * 512:(fi + 1) * 512],
                              pg[:, fi * 512:(fi + 1) * 512],
                              bias_t[:, fi * 3:fi * 3 + 1])
            zt = z_pool.tile([128, 1024], F32, tag="z", name=f"z_{c}")
            for fi in range(2):
                nc.scalar.activation(zt[:, fi * 512:(fi + 1) * 512],
                                     pg[:, 1024 + fi * 512:1024 + (fi + 1) * 512],
                                     SIG, bias=bias_t[:, fi * 3 + 1:fi * 3 + 2])

            rct = rc_pool.tile([128, 4 * 2048], BF16, tag="rc", name=f"rc_{c}")
            pdt = pd_pool.tile([128, 4 * 2048], BF16, tag="pd", name=f"pd_{c}")
            state[c] = {"x": xbt, "hf": hft, "z": zt}
            emit_pair(c, 0, hsc, xrt, rct, pdt)
            emit_pair(c, 1, hsc, xrt, rct, pdt)
            if c == NCH - 1:
                # pre-compute tree L2 for pairs 0+1 so the final tail only
                # waits on the pair 2/3 branch of the tree
                with nc.allow_low_precision(reason="bf16 neighbor sums"):
                    nc.vector.tensor_add(pdt[:, 0:1024], rct[:, 0:1024],
                                         rct[:, 1024:2048])
            if c > 0:
                emit_mid(c)
            emit_pair(c, 2, hsc, xrt, rct, pdt)
            emit_pair(c, 3, hsc, xrt, rct, pdt,
                      act_split=(c == NCH - 1))
            pend[c] = (rct, pdt)

        emit_l23(NCH - 1, skip_l2a=True)
        emit_last_tail(NCH - 1)

    nc.compile()
    return nc


def _prep_inputs(x, h_sum, hs, Wir, bir, Whr, bhr, Wiz, biz, Whz, bhz,
                 Win, bin_, Whn, bhn):
    """Shard + pre-chunk to per-core, per-chunk feature-major HBM layouts."""
    f32 = np.float32
    x = np.asarray(x, f32)
    h = np.asarray(h_sum, f32)
    hs = np.asarray(hs, f32)

    # packed weights, need-ordered; wpL[p, W_OFF[w] + k*256 + f*128 + m]
    # = W[f*128+m, k*128+p]; identity at ID_OFF
    wpack = np.zeros((128, WP_COLS), f32)
    for w, W in (("wir", Wir), ("whr", Whr), ("wiz", Wiz), ("whz", Whz),
                 ("win", Win), ("whn", Whn)):
        WT = np.asarray(W, f32).T  # [in, out]
        for k in range(2):
            wpack[:, W_OFF[w] + k * 256:W_OFF[w] + (k + 1) * 256] = \
                WT[k * 128:(k + 1) * 128, :]
    wpack[:, ID_OFF:ID_OFF + 128] = np.eye(128, dtype=f32)
    wpack_bf = np.ascontiguousarray(wpack.astype(BF_NP))

    b_r = np.asarray(bir, f32) + np.asarray(bhr, f32)
    b_z = np.asarray(biz, f32) + np.asarray(bhz, f32)
    b_n = np.asarray(bin_, f32) + np.asarray(bhn, f32)
    biasp = np.empty((128, 6), f32)
    for f in range(2):
        biasp[:, f * 3 + 0] = b_r[f * 128:(f + 1) * 128]
        biasp[:, f * 3 + 1] = b_z[f * 128:(f + 1) * 128]
        biasp[:, f * 3 + 2] = b_n[f * 128:(f + 1) * 128]

    in_maps = []
    for c in range(M):
        sl = slice(c * BL, (c + 1) * BL)
        # x/h: [BL, 256] -> [ch, b, k, p] -> [ch, p, k, b] -> [ch, 128, 1024]
        xc = x[sl].reshape(NCH, CW, 2, 128).transpose(0, 3, 2, 1)
        hc = h[sl].reshape(NCH, CW, 2, 128).transpose(0, 3, 2, 1)
        xb = np.concatenate([xc.astype(BF_NP).reshape(NCH, 128, 1024),
                             hc.astype(BF_NP).reshape(NCH, 128, 1024)], axis=2)
        # hs: [8, BL, 256] -> [pr, j, ch, b, k, p] -> [ch, p, pr, k, j, b]
        hsc = hs[:, sl, :].reshape(NPAIR, 2, NCH, CW, 2, 128)
        m = {
            "xbL": np.ascontiguousarray(xb),
            "hfL": np.ascontiguousarray(hc).reshape(NCH, 128, 1024),
            "hsL": hsc.transpose(2, 5, 0, 4, 1, 3).astype(BF_NP).reshape(
                NCH, 128, 8192),
            "wpL": wpack_bf,
            "biasp": biasp,
        }
        in_maps.append(m)
    return in_maps


def _run(inputs, trace=False, **trace_kwargs):
    global _cached
    if _cached is None:
        _cached = _build()
    nc = _cached
    in_maps = _prep_inputs(**inputs)
    res = run_bass_kernel_spmd(nc, in_maps, list(range(M)), trace=trace,
                               **trace_kwargs)
    out = np.empty((B, H), np.float32)
    for c in range(M):
        # outL [ch, p, (f b)] -> [ch, b, f, p] -> [BL, 256]
        o = res.results[c]["outL"].reshape(NCH, 128, 2, CW)
        out[c * BL:(c + 1) * BL, :] = o.transpose(0, 3, 2, 1).reshape(BL, 256)
    return out, res


def kernel(**inputs):
    return _run(inputs)[0]


# revision 45
# speedup vs baseline: 1.0201x; 1.0201x over previous
"""GRU-style GNN message-passing kernel for Trainium2 (8 NeuronCores, SPMD).

Reference computation (per node b, features 256, 8 neighbors):
    xr = x @ Wir.T + bir
    hr_n = hs_n @ Whr.T + bhr
    r_n = sigmoid(xr + hr_n)
    z = sigmoid(x @ Wiz.T + biz + h_sum @ Whz.T + bhz)
    s = sum_n r_n * hs_n
    n = tanh(x @ Win.T + bin + s @ Whn.T + bhn)
    out = (1 - z) * n + z * h_sum

Strategy: data-parallel over B=32768 across 8 cores (4096 rows each),
8 batch-chunks of 512 per core, feature-major on-chip layout
([256 features = 2 partition chunks of 128, batch free dim]).

The schedule is built around keeping the PE matmul stream dense (any PE
idle gap re-engages the HAM clock throttle and halves the PE clock):

  - Host-side pre-chunked HBM layouts: every DMA is a plain 2D copy
    with 2-16KB contiguous runs; one hs DMA per chunk (split per-pair
    for chunk 0 so pair 0 lands early); x and bf16-h_sum packed in one
    tensor; all weights in one need-ordered pack, DMA'd in two pieces.
  - Per chunk PE stream: a 12-matmul [xr | z-gate] front block (the PE
    runway that covers ACT's end-of-previous-chunk lag), r-matmul
    pairs 0-1, the deferred n-gate of chunk c-1, pairs 2-3.  The DVE
    product tree of chunk c completes at the start of chunk c+1 (which
    also fills DVE's would-be idle/re-throttle gap), so PE never waits
    on it.
  - Each neighbor pair accumulates in one [128,2048] PSUM tile (4
    interleaved 512-wide regions: Whr k0/k1 matmuls + an identity
    matmul that adds the shared xr) and drains with a single wide
    sigmoid ACTIVATE.
  - DVE runs everything alias-free in the packed bf16 2x mode:
    products per pair, the add tree, and the final combine
    out = n + z*(h-n) in fp32 (h_sum kept fp32 for the dominant term).
"""

import sys
import numpy as np
from contextlib import ExitStack

sys.path.insert(0, "/opt/trn_rl_repo")

import ml_dtypes
import concourse.bacc as bacc
import concourse.tile as tile
from concourse import mybir
from concourse.bass_utils import run_bass_kernel_spmd

F32 = mybir.dt.float32
BF16 = mybir.dt.bfloat16
BF_NP = ml_dtypes.bfloat16

N_NEIGH, B, IN, H = 8, 32768, 256, 256
M = 8                    # cores
BL = B // M              # rows per core (4096)
NCH = 8                  # batch chunks per core
CW = BL // NCH           # chunk width (512)
NPAIR = N_NEIGH // 2     # neighbor pairs (4)

_cached = None  # compiled program, reused across kernel() calls

SIG = mybir.ActivationFunctionType.Sigmoid
TANH = mybir.ActivationFunctionType.Tanh

# weight pack column offsets (need-ordered: xr gate, z gate, r pairs, n)
W_OFF = {"wir": 0, "wiz": 512, "whz": 1024, "whr": 1536, "win": 2176,
         "whn": 2688}
ID_OFF = 2048
WP_COLS = 3200
WP_SPLIT = 1536  # piece A: wir/wiz/whz; piece B: whr/id/win/whn


def _build():
    nc = bacc.Bacc("TRN2", target_bir_lowering=False, debug=False, num_devices=M)

    # xbL packs x (cols 0:1024) and h_sum-bf16 (cols 1024:2048) per chunk
    xbL = nc.dram_tensor("xbL", [NCH, 128, 2048], BF16, kind="ExternalInput").ap()
    hfL = nc.dram_tensor("hfL", [NCH, 128, 1024], F32, kind="ExternalInput").ap()
    hsL = nc.dram_tensor("hsL", [NCH, 128, 8192], BF16,
                         kind="ExternalInput").ap()
    wpL = nc.dram_tensor("wpL", [128, WP_COLS], BF16, kind="ExternalInput").ap()
    # bias pack: col f*3+j holds feature-chunk f of (b_r, b_z, b_n)[j]
    biasp = nc.dram_tensor("biasp", [128, 6], F32, kind="ExternalInput").ap()
    outL = nc.dram_tensor("outL", [NCH, 128, 1024], F32, kind="ExternalOutput").ap()

    with tile.TileContext(nc) as tc, ExitStack() as ctx:
        cpool = ctx.enter_context(tc.tile_pool(name="const", bufs=1))
        x_pool = ctx.enter_context(tc.tile_pool(name="x", bufs=3))
        hf_pool = ctx.enter_context(tc.tile_pool(name="hf", bufs=3))
        hs_pool = ctx.enter_context(tc.tile_pool(name="hs", bufs=3))
        xr_pool = ctx.enter_context(tc.tile_pool(name="xr", bufs=2))
        z_pool = ctx.enter_context(tc.tile_pool(name="z", bufs=2))
        rc_pool = ctx.enter_context(tc.tile_pool(name="rc", bufs=2))
        pd_pool = ctx.enter_context(tc.tile_pool(name="pd", bufs=2))
        s_pool = ctx.enter_context(tc.tile_pool(name="s", bufs=2))
        n_pool = ctx.enter_context(tc.tile_pool(name="n", bufs=2))
        d_pool = ctx.enter_context(tc.tile_pool(name="d", bufs=2))
        o_pool = ctx.enter_context(tc.tile_pool(name="o", bufs=2))
        pp_pool = ctx.enter_context(tc.tile_pool(name="pp", bufs=2, space="PSUM"))

        # --- constants: weight pack in two need-ordered DMAs + biases ---
        wp_t = cpool.tile([128, WP_COLS], BF16, tag="wp", name="wp_t")
        nc.sync.dma_start(out=wp_t[:, 0:WP_SPLIT], in_=wpL[:, 0:WP_SPLIT])
        bias_t = cpool.tile([128, 6], F32, tag="biasp", name="bias_t")
        nc.sync.dma_start(out=bias_t[:, :], in_=biasp[:, :])
        # piece B (whr/id/win/whn) is DMA'd inside chunk 0, after the data
        # the very first matmuls need, so PE starts ~6us earlier

        # warm-up: the PE HAM clock-gate needs ~3.4us of sustained activity
        # to lift the 1.2GHz cold throttle. Run dummy matmuls on a zeroed
        # tile during the startup DMA window so real work starts at 2.4GHz.
        wu_t = cpool.tile([128, 128], BF16, tag="wu", name="wu_t")
        nc.vector.memset(wu_t[:, :], 0)
        pwu = pp_pool.tile([128, 2048], F32, tag="pp", name="pwu")
        for i in range(44):
            nc.tensor.matmul(pwu[:, (i % 4) * 512:(i % 4) * 512 + 128],
                             wu_t[:, :], wu_t[:, :], start=True, stop=True)

        def wcol(w, k, f):  # stationary [128,128] for weight w, k-chunk, f-chunk
            off = W_OFF[w] + k * 256 + f * 128
            return wp_t[:, off:off + 128]

        id_t = wp_t[:, ID_OFF:ID_OFF + 128]

        state = {}  # chunk -> tiles needed by the deferred n-gate/combine

        def emit_mid(c):
            """Between pairs 1 and 2 of chunk c: the deferred n-gate of
            chunk c-1, then its combine on DVE + store."""
            st = state.pop(c - 1)
            pn = pp_pool.tile([128, 2048], F32, tag="pp", name=f"pn_{c - 1}")
            for fi in range(2):
                o = pn[:, fi * 512:(fi + 1) * 512]
                nc.tensor.matmul(o, wcol("win", 0, fi), st["x"][:, 0:512],
                                 start=True, stop=False)
                nc.tensor.matmul(o, wcol("win", 1, fi), st["x"][:, 512:1024],
                                 start=False, stop=False)
                nc.tensor.matmul(o, wcol("whn", 0, fi), st["s"][:, 0:512],
                                 start=False, stop=False)
                nc.tensor.matmul(o, wcol("whn", 1, fi), st["s"][:, 512:1024],
                                 start=False, stop=True)
            nt = n_pool.tile([128, 1024], F32, tag="n", name=f"n_{c - 1}")
            for fi in range(2):
                nc.scalar.activation(nt[:, fi * 512:(fi + 1) * 512],
                                     pn[:, fi * 512:(fi + 1) * 512], TANH,
                                     bias=bias_t[:, fi * 3 + 2:fi * 3 + 3])
            # out = n + z * (h - n) on DVE, then store
            dt_ = d_pool.tile([128, 1024], F32, tag="d", name=f"d_{c - 1}")
            nc.vector.tensor_sub(dt_[:, :], st["hf"][:, :], nt[:, :])
            nc.vector.tensor_mul(dt_[:, :], st["z"][:, :], dt_[:, :])
            ot = o_pool.tile([128, 1024], F32, tag="o", name=f"o_{c - 1}")
            nc.vector.tensor_add(ot[:, :], nt[:, :], dt_[:, :])
            nc.sync.dma_start(out=outL[c - 1], in_=ot[:, :])

        def emit_last_tail(cc):
            """n-gate + combine for the final chunk, f-split to shorten the
            end-of-kernel serial chain."""
            st = state.pop(cc)
            pn = pp_pool.tile([128, 2048], F32, tag="pp", name=f"pn_{cc}")
            nt = n_pool.tile([128, 1024], F32, tag="n", name=f"n_{cc}")
            dt_ = d_pool.tile([128, 1024], F32, tag="d", name=f"d_{cc}")
            ot = o_pool.tile([128, 1024], F32, tag="o", name=f"o_{cc}")
            for fi in range(2):
                o = pn[:, fi * 512:(fi + 1) * 512]
                nc.tensor.matmul(o, wcol("win", 0, fi), st["x"][:, 0:512],
                                 start=True, stop=False)
                nc.tensor.matmul(o, wcol("win", 1, fi), st["x"][:, 512:1024],
                                 start=False, stop=False)
                nc.tensor.matmul(o, wcol("whn", 0, fi), st["s"][:, 0:512],
                                 start=False, stop=False)
                nc.tensor.matmul(o, wcol("whn", 1, fi), st["s"][:, 512:1024],
                                 start=False, stop=True)
                nc.scalar.activation(nt[:, fi * 512:(fi + 1) * 512],
                                     pn[:, fi * 512:(fi + 1) * 512], TANH,
                                     bias=bias_t[:, fi * 3 + 2:fi * 3 + 3])
                s_ = slice(fi * 512, (fi + 1) * 512)
                nc.vector.tensor_sub(dt_[:, s_], st["hf"][:, s_], nt[:, s_])
                nc.vector.tensor_mul(dt_[:, s_], st["z"][:, s_], dt_[:, s_])
                nc.vector.tensor_add(ot[:, s_], nt[:, s_], dt_[:, s_])
                nc.sync.dma_start(out=outL[cc][:, s_], in_=ot[:, s_])

        def emit_pair(c, p, hsc, xrt, rct, pdt, act_split=False):
            base = p * 2048
            # one PSUM tile per pair, (f, j, b) layout; four interleaved
            # 512-wide accumulation regions. whr matmuls first, the xr
            # identity adds last so xr is never waited on.
            pra = pp_pool.tile([128, 2048], F32, tag="pp", name=f"pr{p}_{c}")
            for fi in range(2):
                oj0 = pra[:, fi * 1024:fi * 1024 + 512]
                oj1 = pra[:, fi * 1024 + 512:fi * 1024 + 1024]
                nc.tensor.matmul(oj0, wcol("whr", 0, fi),
                                 hsc[:, base:base + 512],
                                 start=True, stop=False)
                nc.tensor.matmul(oj1, wcol("whr", 0, fi),
                                 hsc[:, base + 512:base + 1024],
                                 start=True, stop=False)
                nc.tensor.matmul(oj0, wcol("whr", 1, fi),
                                 hsc[:, base + 1024:base + 1536],
                                 start=False, stop=False)
                nc.tensor.matmul(oj1, wcol("whr", 1, fi),
                                 hsc[:, base + 1536:base + 2048],
                                 start=False, stop=False)
            for fi in range(2):
                nc.tensor.matmul(pra[:, fi * 1024:fi * 1024 + 512], id_t,
                                 xrt[:, fi * 512:(fi + 1) * 512],
                                 start=False, stop=True)
                nc.tensor.matmul(pra[:, fi * 1024 + 512:fi * 1024 + 1024],
                                 id_t, xrt[:, fi * 512:(fi + 1) * 512],
                                 start=False, stop=True)
            if act_split:
                # f-split activation/product path (used for the final pair
                # of the final chunk to shorten the tail chain)
                for fi in range(2):
                    hb = base + fi * 1024
                    nc.scalar.activation(rct[:, hb:hb + 1024],
                                         pra[:, fi * 1024:(fi + 1) * 1024], SIG)
                    nc.vector.tensor_mul(pdt[:, hb:hb + 1024],
                                         rct[:, hb:hb + 1024],
                                         hsc[:, hb:hb + 1024])
                    with nc.allow_low_precision(reason="bf16 neighbor sums"):
                        nc.vector.tensor_add(
                            rct[:, p * 1024 + fi * 512:p * 1024 + fi * 512 + 512],
                            pdt[:, hb:hb + 512],
                            pdt[:, hb + 512:hb + 1024])
            else:
                # r for pair p, both f chunks in one activation
                nc.scalar.activation(rct[:, base:base + 2048], pra[:, :], SIG)
                # products r*hs for the whole pair block (alias-free so the
                # DVE packed bf16 mode stays eligible)
                blk = slice(base, base + 2048)
                nc.vector.tensor_mul(pdt[:, blk], rct[:, blk], hsc[:, blk])
                # tree level 1: j0 + j1 per f chunk -> rc cols [p*1024, +1024)
                with nc.allow_low_precision(reason="bf16 neighbor sums"):
                    for fi in range(2):
                        nc.vector.tensor_add(
                            rct[:, p * 1024 + fi * 512:p * 1024 + fi * 512 + 512],
                            pdt[:, base + fi * 1024:base + fi * 1024 + 512],
                            pdt[:, base + fi * 1024 + 512:base + fi * 1024 + 1024])

        pend = {}  # chunk -> (rct, pdt) awaiting tree levels 2+3

        def emit_l23(cc, skip_l2a=False):
            rct, pdt = pend.pop(cc)
            sct = s_pool.tile([128, 1024], BF16, tag="s", name=f"s_{cc}")
            with nc.allow_low_precision(reason="bf16 neighbor sums"):
                if not skip_l2a:
                    nc.vector.tensor_add(pdt[:, 0:1024], rct[:, 0:1024],
                                         rct[:, 1024:2048])
                nc.vector.tensor_add(pdt[:, 1024:2048], rct[:, 2048:3072],
                                     rct[:, 3072:4096])
                nc.vector.tensor_add(sct[:, :], pdt[:, 0:1024],
                                     pdt[:, 1024:2048])
            state[cc]["s"] = sct

        for c in range(NCH):
            # --- input DMAs (plain 2D copies, 4-16KB contiguous runs);
            #     chunk 0's hs comes in per-pair so pair0 lands early ---
            xbt = x_pool.tile([128, 2048], BF16, tag="x", name=f"x_{c}")
            nc.sync.dma_start(out=xbt[:, :], in_=xbL[c])
            hsc = hs_pool.tile([128, 8192], BF16, tag="hs", name=f"hs_{c}")
            if c == 0:
                nc.sync.dma_start(out=hsc[:, 0:2048], in_=hsL[c][:, 0:2048])
                nc.sync.dma_start(out=wp_t[:, WP_SPLIT:WP_COLS],
                                  in_=wpL[:, WP_SPLIT:WP_COLS])
                for p in range(1, NPAIR):
                    nc.sync.dma_start(out=hsc[:, p * 2048:(p + 1) * 2048],
                                      in_=hsL[c][:, p * 2048:(p + 1) * 2048])
            else:
                nc.sync.dma_start(out=hsc[:, :], in_=hsL[c])
            hft = hf_pool.tile([128, 1024], F32, tag="hf", name=f"hf_{c}")
            nc.sync.dma_start(out=hft[:, :], in_=hfL[c])

            # tree tail of the previous chunk opens the DVE stream here,
            # filling what would otherwise be a DVE idle (re-throttle) gap
            if c > 0:
                emit_l23(c - 1)

            # --- chunk-front gates in one PSUM tile: xr = Wir@x + b_r
            #     (cols 0:1024) and the z pre-act (cols 1024:2048); this
            #     12-matmul block is the PE runway that covers ACT's
            #     end-of-previous-chunk lag ---
            pg = pp_pool.tile([128, 2048], F32, tag="pp", name=f"pg_{c}")
            for fi in range(2):
                o = pg[:, fi * 512:(fi + 1) * 512]
                nc.tensor.matmul(o, wcol("wir", 0, fi), xbt[:, 0:512],
                                 start=True, stop=False)
                nc.tensor.matmul(o, wcol("wir", 1, fi), xbt[:, 512:1024],
                                 start=False, stop=True)
            for fi in range(2):
                o = pg[:, 1024 + fi * 512:1024 + (fi + 1) * 512]
                nc.tensor.matmul(o, wcol("wiz", 0, fi), xbt[:, 0:512],
                                 start=True, stop=False)
                nc.tensor.matmul(o, wcol("wiz", 1, fi), xbt[:, 512:1024],
                                 start=False, stop=False)
                nc.tensor.matmul(o, wcol("whz", 0, fi), xbt[:, 1024:1536],
                                 start=False, stop=False)
                nc.tensor.matmul(o, wcol("whz", 1, fi), xbt[:, 1536:2048],
                                 start=False, stop=True)
            xrt = xr_pool.tile([128, 1024], BF16, tag="xr", name=f"xr_{c}")
            for fi in range(2):
                nc.scalar.add(xrt[:, fi * 512:(fi + 1) * 512],
                              pg[:, fi * 512:(fi + 1) * 512],
                              bias_t[:, fi * 3:fi * 3 + 1])
            zt = z_pool.tile([128, 1024], F32, tag="z", name=f"z_{c}")
            for fi in range(2):
                nc.scalar.activation(zt[:, fi * 512:(fi + 1) * 512],
                                     pg[:, 1024 + fi * 512:1024 + (fi + 1) * 512],
                                     SIG, bias=bias_t[:, fi * 3 + 1:fi * 3 + 2])

            rct = rc_pool.tile([128, 4 * 2048], BF16, tag="rc", name=f"rc_{c}")
            pdt = pd_pool.tile([128, 4 * 2048], BF16, tag="pd", name=f"pd_{c}")
            state[c] = {"x": xbt, "hf": hft, "z": zt}
            emit_pair(c, 0, hsc, xrt, rct, pdt)
            emit_pair(c, 1, hsc, xrt, rct, pdt)
            if c == NCH - 1:
                # pre-compute tree L2 for pairs 0+1 so the final tail only
                # waits on the pair 2/3 branch of the tree
                with nc.allow_low_precision(reason="bf16 neighbor sums"):
                    nc.vector.tensor_add(pdt[:, 0:1024], rct[:, 0:1024],
                                         rct[:, 1024:2048])
            if c > 0:
                emit_mid(c)
            emit_pair(c, 2, hsc, xrt, rct, pdt)
            emit_pair(c, 3, hsc, xrt, rct, pdt,
                      act_split=(c == NCH - 1))
            pend[c] = (rct, pdt)

        emit_l23(NCH - 1, skip_l2a=True)
        emit_last_tail(NCH - 1)

    nc.compile()
    return nc


def _prep_inputs(x, h_sum, hs, Wir, bir, Whr, bhr, Wiz, biz, Whz, bhz,
                 Win, bin_, Whn, bhn):
    """Shard + pre-chunk to per-core, per-chunk feature-major HBM layouts."""
    f32 = np.float32
    x = np.asarray(x, f32)
    h = np.asarray(h_sum, f32)
    hs = np.asarray(hs, f32)

    # packed weights, need-ordered; wpL[p, W_OFF[w] + k*256 + f*128 + m]
    # = W[f*128+m, k*128+p]; identity at ID_OFF
    wpack = np.zeros((128, WP_COLS), f32)
    for w, W in (("wir", Wir), ("whr", Whr), ("wiz", Wiz), ("whz", Whz),
                 ("win", Win), ("whn", Whn)):
        WT = np.asarray(W, f32).T  # [in, out]
        for k in range(2):
            wpack[:, W_OFF[w] + k * 256:W_OFF[w] + (k + 1) * 256] = \
                WT[k * 128:(k + 1) * 128, :]
    wpack[:, ID_OFF:ID_OFF + 128] = np.eye(128, dtype=f32)
    wpack_bf = np.ascontiguousarray(wpack.astype(BF_NP))

    b_r = np.asarray(bir, f32) + np.asarray(bhr, f32)
    b_z = np.asarray(biz, f32) + np.asarray(bhz, f32)
    b_n = np.asarray(bin_, f32) + np.asarray(bhn, f32)
    biasp = np.empty((128, 6), f32)
    for f in range(2):
        biasp[:, f * 3 + 0] = b_r[f * 128:(f + 1) * 128]
        biasp[:, f * 3 + 1] = b_z[f * 128:(f + 1) * 128]
        biasp[:, f * 3 + 2] = b_n[f * 128:(f + 1) * 128]

    in_maps = []
    for c in range(M):
        sl = slice(c * BL, (c + 1) * BL)
        # x/h: [BL, 256] -> [ch, b, k, p] -> [ch, p, k, b] -> [ch, 128, 1024]
        xc = x[sl].reshape(NCH, CW, 2, 128).transpose(0, 3, 2, 1)
        hc = h[sl].reshape(NCH, CW, 2, 128).transpose(0, 3, 2, 1)
        xb = np.concatenate([xc.astype(BF_NP).reshape(NCH, 128, 1024),
                             hc.astype(BF_NP).reshape(NCH, 128, 1024)], axis=2)
        # hs: [8, BL, 256] -> [pr, j, ch, b, k, p] -> [ch, p, pr, k, j, b]
        hsc = hs[:, sl, :].reshape(NPAIR, 2, NCH, CW, 2, 128)
        m = {
            "xbL": np.ascontiguousarray(xb),
            "hfL": np.ascontiguousarray(hc).reshape(NCH, 128, 1024),
            "hsL": hsc.transpose(2, 5, 0, 4, 1, 3).astype(BF_NP).reshape(
                NCH, 128, 8192),
            "wpL": wpack_bf,
            "biasp": biasp,
        }
        in_maps.append(m)
    return in_maps


def _run(inputs, trace=False, **trace_kwargs):
    global _cached
    if _cached is None:
        _cached = _build()
    nc = _cached
    in_maps = _prep_inputs(**inputs)
    res = run_bass_kernel_spmd(nc, in_maps, list(range(M)), trace=trace,
                               **trace_kwargs)
    out = np.empty((B, H), np.float32)
    for c in range(M):
        # outL [ch, p, (f b)] -> [ch, b, f, p] -> [BL, 256]
        o = res.results[c]["outL"].reshape(NCH, 128, 2, CW)
        out[c * BL:(c + 1) * BL, :] = o.transpose(0, 3, 2, 1).reshape(BL, 256)
    return out, res


def kernel(**inputs):
    return _run(inputs)[0]


# revision 46
# speedup vs baseline: 1.0230x; 1.0029x over previous
"""GRU-style GNN message-passing kernel for Trainium2 (8 NeuronCores, SPMD).

Reference computation (per node b, features 256, 8 neighbors):
    xr = x @ Wir.T + bir
    hr_n = hs_n @ Whr.T + bhr
    r_n = sigmoid(xr + hr_n)
    z = sigmoid(x @ Wiz.T + biz + h_sum @ Whz.T + bhz)
    s = sum_n r_n * hs_n
    n = tanh(x @ Win.T + bin + s @ Whn.T + bhn)
    out = (1 - z) * n + z * h_sum

Strategy: data-parallel over B=32768 across 8 cores (4096 rows each),
8 batch-chunks of 512 per core, feature-major on-chip layout
([256 features = 2 partition chunks of 128, batch free dim]).

The schedule is built around keeping the PE matmul stream dense (any PE
idle gap re-engages the HAM clock throttle and halves the PE clock):

  - Host-side pre-chunked HBM layouts: every DMA is a plain 2D copy
    with 2-16KB contiguous runs; one hs DMA per chunk (split per-pair
    for chunk 0 so pair 0 lands early); x and bf16-h_sum packed in one
    tensor; all weights in one need-ordered pack, DMA'd in two pieces.
  - Per chunk PE stream: a 12-matmul [xr | z-gate] front block (the PE
    runway that covers ACT's end-of-previous-chunk lag), r-matmul
    pairs 0-1, the deferred n-gate of chunk c-1, pairs 2-3.  The DVE
    product tree of chunk c completes at the start of chunk c+1 (which
    also fills DVE's would-be idle/re-throttle gap), so PE never waits
    on it.
  - Each neighbor pair accumulates in one [128,2048] PSUM tile (4
    interleaved 512-wide regions: Whr k0/k1 matmuls + an identity
    matmul that adds the shared xr) and drains with a single wide
    sigmoid ACTIVATE.
  - DVE runs everything alias-free in the packed bf16 2x mode:
    products per pair, the add tree, and the final combine
    out = n + z*(h-n) in fp32 (h_sum kept fp32 for the dominant term).
"""

import sys
import numpy as np
from contextlib import ExitStack

sys.path.insert(0, "/opt/trn_rl_repo")

import ml_dtypes
import concourse.bacc as bacc
import concourse.tile as tile
from concourse import mybir
from concourse.bass_utils import run_bass_kernel_spmd

F32 = mybir.dt.float32
BF16 = mybir.dt.bfloat16
BF_NP = ml_dtypes.bfloat16

N_NEIGH, B, IN, H = 8, 32768, 256, 256
M = 8                    # cores
BL = B // M              # rows per core (4096)
NCH = 8                  # batch chunks per core
CW = BL // NCH           # chunk width (512)
NPAIR = N_NEIGH // 2     # neighbor pairs (4)

_cached = None  # compiled program, reused across kernel() calls

SIG = mybir.ActivationFunctionType.Sigmoid
TANH = mybir.ActivationFunctionType.Tanh

# weight pack column offsets (need-ordered: xr gate, z gate, r pairs, n)
W_OFF = {"wir": 0, "wiz": 512, "whz": 1024, "whr": 1536, "win": 2176,
         "whn": 2688}
ID_OFF = 2048
WP_COLS = 3200
WP_SPLIT = 1536  # piece A: wir/wiz/whz; piece B: whr/id/win/whn


def _build():
    nc = bacc.Bacc("TRN2", target_bir_lowering=False, debug=False, num_devices=M)

    # xbL packs x (cols 0:1024) and h_sum-bf16 (cols 1024:2048) per chunk
    xbL = nc.dram_tensor("xbL", [NCH, 128, 2048], BF16, kind="ExternalInput").ap()
    hfL = nc.dram_tensor("hfL", [NCH, 128, 1024], F32, kind="ExternalInput").ap()
    hsL = nc.dram_tensor("hsL", [NCH, 128, 8192], BF16,
                         kind="ExternalInput").ap()
    wpL = nc.dram_tensor("wpL", [128, WP_COLS], BF16, kind="ExternalInput").ap()
    # bias pack: col f*3+j holds feature-chunk f of (b_r, b_z, b_n)[j]
    biasp = nc.dram_tensor("biasp", [128, 6], F32, kind="ExternalInput").ap()
    outL = nc.dram_tensor("outL", [NCH, 128, 1024], F32, kind="ExternalOutput").ap()

    with tile.TileContext(nc) as tc, ExitStack() as ctx:
        cpool = ctx.enter_context(tc.tile_pool(name="const", bufs=1))
        x_pool = ctx.enter_context(tc.tile_pool(name="x", bufs=3))
        hf_pool = ctx.enter_context(tc.tile_pool(name="hf", bufs=3))
        hs_pool = ctx.enter_context(tc.tile_pool(name="hs", bufs=3))
        xr_pool = ctx.enter_context(tc.tile_pool(name="xr", bufs=2))
        z_pool = ctx.enter_context(tc.tile_pool(name="z", bufs=2))
        rc_pool = ctx.enter_context(tc.tile_pool(name="rc", bufs=2))
        pd_pool = ctx.enter_context(tc.tile_pool(name="pd", bufs=2))
        s_pool = ctx.enter_context(tc.tile_pool(name="s", bufs=2))
        n_pool = ctx.enter_context(tc.tile_pool(name="n", bufs=2))
        d_pool = ctx.enter_context(tc.tile_pool(name="d", bufs=2))
        o_pool = ctx.enter_context(tc.tile_pool(name="o", bufs=2))
        pp_pool = ctx.enter_context(tc.tile_pool(name="pp", bufs=2, space="PSUM"))

        # --- constants: weight pack in two need-ordered DMAs + biases ---
        wp_t = cpool.tile([128, WP_COLS], BF16, tag="wp", name="wp_t")
        nc.sync.dma_start(out=wp_t[:, 0:WP_SPLIT], in_=wpL[:, 0:WP_SPLIT])
        bias_t = cpool.tile([128, 6], F32, tag="biasp", name="bias_t")
        nc.sync.dma_start(out=bias_t[:, :], in_=biasp[:, :])
        # piece B (whr/id/win/whn) is DMA'd inside chunk 0, after the data
        # the very first matmuls need, so PE starts ~6us earlier

        # warm-up: the PE HAM clock-gate needs ~3.4us of sustained activity
        # to lift the 1.2GHz cold throttle. Run dummy matmuls on a zeroed
        # tile during the startup DMA window so real work starts at 2.4GHz.
        wu_t = cpool.tile([128, 128], BF16, tag="wu", name="wu_t")
        nc.vector.memset(wu_t[:, :], 0)
        pwu = pp_pool.tile([128, 2048], F32, tag="pp", name="pwu")
        for i in range(44):
            nc.tensor.matmul(pwu[:, (i % 4) * 512:(i % 4) * 512 + 128],
                             wu_t[:, :], wu_t[:, :], start=True, stop=True)

        def wcol(w, k, f):  # stationary [128,128] for weight w, k-chunk, f-chunk
            off = W_OFF[w] + k * 256 + f * 128
            return wp_t[:, off:off + 128]

        id_t = wp_t[:, ID_OFF:ID_OFF + 128]

        state = {}  # chunk -> tiles needed by the deferred n-gate/combine

        def emit_mid(c):
            """Between pairs 1 and 2 of chunk c: the deferred n-gate of
            chunk c-1, then its combine on DVE + store."""
            st = state.pop(c - 1)
            pn = pp_pool.tile([128, 2048], F32, tag="pp", name=f"pn_{c - 1}")
            for fi in range(2):
                o = pn[:, fi * 512:(fi + 1) * 512]
                nc.tensor.matmul(o, wcol("win", 0, fi), st["x"][:, 0:512],
                                 start=True, stop=False)
                nc.tensor.matmul(o, wcol("win", 1, fi), st["x"][:, 512:1024],
                                 start=False, stop=False)
                nc.tensor.matmul(o, wcol("whn", 0, fi), st["s"][:, 0:512],
                                 start=False, stop=False)
                nc.tensor.matmul(o, wcol("whn", 1, fi), st["s"][:, 512:1024],
                                 start=False, stop=True)
            nt = n_pool.tile([128, 1024], F32, tag="n", name=f"n_{c - 1}")
            for fi in range(2):
                nc.scalar.activation(nt[:, fi * 512:(fi + 1) * 512],
                                     pn[:, fi * 512:(fi + 1) * 512], TANH,
                                     bias=bias_t[:, fi * 3 + 2:fi * 3 + 3])
            # out = n + z * (h - n) on DVE, then store
            dt_ = d_pool.tile([128, 1024], F32, tag="d", name=f"d_{c - 1}")
            nc.vector.tensor_sub(dt_[:, :], st["hf"][:, :], nt[:, :])
            nc.vector.tensor_mul(dt_[:, :], st["z"][:, :], dt_[:, :])
            ot = o_pool.tile([128, 1024], F32, tag="o", name=f"o_{c - 1}")
            nc.vector.tensor_add(ot[:, :], nt[:, :], dt_[:, :])
            nc.sync.dma_start(out=outL[c - 1], in_=ot[:, :])

        def emit_last_tail(cc):
            """n-gate + combine for the final chunk, f-split to shorten the
            end-of-kernel serial chain."""
            st = state.pop(cc)
            pn = pp_pool.tile([128, 2048], F32, tag="pp", name=f"pn_{cc}")
            nt = n_pool.tile([128, 1024], F32, tag="n", name=f"n_{cc}")
            dt_ = d_pool.tile([128, 1024], F32, tag="d", name=f"d_{cc}")
            ot = o_pool.tile([128, 1024], F32, tag="o", name=f"o_{cc}")
            for fi in range(2):
                o = pn[:, fi * 512:(fi + 1) * 512]
                nc.tensor.matmul(o, wcol("win", 0, fi), st["x"][:, 0:512],
                                 start=True, stop=False)
                nc.tensor.matmul(o, wcol("win", 1, fi), st["x"][:, 512:1024],
                                 start=False, stop=False)
                nc.tensor.matmul(o, wcol("whn", 0, fi), st["s"][:, 0:512],
                                 start=False, stop=False)
                nc.tensor.matmul(o, wcol("whn", 1, fi), st["s"][:, 512:1024],
                                 start=False, stop=True)
                nc.scalar.activation(nt[:, fi * 512:(fi + 1) * 512],
                                     pn[:, fi * 512:(fi + 1) * 512], TANH,
                                     bias=bias_t[:, fi * 3 + 2:fi * 3 + 3])
                s_ = slice(fi * 512, (fi + 1) * 512)
                nc.vector.tensor_sub(dt_[:, s_], st["hf"][:, s_], nt[:, s_])
                nc.vector.tensor_mul(dt_[:, s_], st["z"][:, s_], dt_[:, s_])
                nc.vector.tensor_add(ot[:, s_], nt[:, s_], dt_[:, s_])
                nc.sync.dma_start(out=outL[cc][:, s_], in_=ot[:, s_])

        def emit_pair(c, p, hsc, xrt, rct, pdt, act_split=False):
            base = p * 2048
            # one PSUM tile per pair, (f, j, b) layout; four interleaved
            # 512-wide accumulation regions. whr matmuls first, the xr
            # identity adds last so xr is never waited on.
            pra = pp_pool.tile([128, 2048], F32, tag="pp", name=f"pr{p}_{c}")
            for fi in range(2):
                oj0 = pra[:, fi * 1024:fi * 1024 + 512]
                oj1 = pra[:, fi * 1024 + 512:fi * 1024 + 1024]
                nc.tensor.matmul(oj0, wcol("whr", 0, fi),
                                 hsc[:, base:base + 512],
                                 start=True, stop=False)
                nc.tensor.matmul(oj1, wcol("whr", 0, fi),
                                 hsc[:, base + 512:base + 1024],
                                 start=True, stop=False)
                nc.tensor.matmul(oj0, wcol("whr", 1, fi),
                                 hsc[:, base + 1024:base + 1536],
                                 start=False, stop=False)
                nc.tensor.matmul(oj1, wcol("whr", 1, fi),
                                 hsc[:, base + 1536:base + 2048],
                                 start=False, stop=False)
            for fi in range(2):
                nc.tensor.matmul(pra[:, fi * 1024:fi * 1024 + 512], id_t,
                                 xrt[:, fi * 512:(fi + 1) * 512],
                                 start=False, stop=True)
                nc.tensor.matmul(pra[:, fi * 1024 + 512:fi * 1024 + 1024],
                                 id_t, xrt[:, fi * 512:(fi + 1) * 512],
                                 start=False, stop=True)
            if act_split:
                # f-split activation/product path (used for the final pair
                # of the final chunk to shorten the tail chain)
                for fi in range(2):
                    hb = base + fi * 1024
                    nc.scalar.activation(rct[:, hb:hb + 1024],
                                         pra[:, fi * 1024:(fi + 1) * 1024], SIG)
                    nc.vector.tensor_mul(pdt[:, hb:hb + 1024],
                                         rct[:, hb:hb + 1024],
                                         hsc[:, hb:hb + 1024])
                    with nc.allow_low_precision(reason="bf16 neighbor sums"):
                        nc.vector.tensor_add(
                            rct[:, p * 1024 + fi * 512:p * 1024 + fi * 512 + 512],
                            pdt[:, hb:hb + 512],
                            pdt[:, hb + 512:hb + 1024])
            else:
                # r for pair p, both f chunks in one activation
                nc.scalar.activation(rct[:, base:base + 2048], pra[:, :], SIG)
                # products r*hs for the whole pair block (alias-free so the
                # DVE packed bf16 mode stays eligible)
                blk = slice(base, base + 2048)
                nc.vector.tensor_mul(pdt[:, blk], rct[:, blk], hsc[:, blk])
                # tree level 1: j0 + j1 per f chunk -> rc cols [p*1024, +1024)
                with nc.allow_low_precision(reason="bf16 neighbor sums"):
                    for fi in range(2):
                        nc.vector.tensor_add(
                            rct[:, p * 1024 + fi * 512:p * 1024 + fi * 512 + 512],
                            pdt[:, base + fi * 1024:base + fi * 1024 + 512],
                            pdt[:, base + fi * 1024 + 512:base + fi * 1024 + 1024])

        pend = {}  # chunk -> (rct, pdt) awaiting tree levels 2+3

        def emit_l23(cc, skip_l2a=False):
            rct, pdt = pend.pop(cc)
            sct = s_pool.tile([128, 1024], BF16, tag="s", name=f"s_{cc}")
            with nc.allow_low_precision(reason="bf16 neighbor sums"):
                if not skip_l2a:
                    nc.vector.tensor_add(pdt[:, 0:1024], rct[:, 0:1024],
                                         rct[:, 1024:2048])
                nc.vector.tensor_add(pdt[:, 1024:2048], rct[:, 2048:3072],
                                     rct[:, 3072:4096])
                nc.vector.tensor_add(sct[:, :], pdt[:, 0:1024],
                                     pdt[:, 1024:2048])
            state[cc]["s"] = sct

        for c in range(NCH):
            # --- input DMAs (plain 2D copies, 4-16KB contiguous runs);
            #     chunk 0's hs comes in per-pair so pair0 lands early ---
            xbt = x_pool.tile([128, 2048], BF16, tag="x", name=f"x_{c}")
            nc.sync.dma_start(out=xbt[:, :], in_=xbL[c])
            hsc = hs_pool.tile([128, 8192], BF16, tag="hs", name=f"hs_{c}")
            if c == 0:
                nc.sync.dma_start(out=hsc[:, 0:2048], in_=hsL[c][:, 0:2048])
                nc.sync.dma_start(out=wp_t[:, WP_SPLIT:WP_COLS],
                                  in_=wpL[:, WP_SPLIT:WP_COLS])
                for p in range(1, NPAIR):
                    nc.sync.dma_start(out=hsc[:, p * 2048:(p + 1) * 2048],
                                      in_=hsL[c][:, p * 2048:(p + 1) * 2048])
            else:
                nc.sync.dma_start(out=hsc[:, :], in_=hsL[c])
            hft = hf_pool.tile([128, 1024], F32, tag="hf", name=f"hf_{c}")
            nc.sync.dma_start(out=hft[:, :], in_=hfL[c])

            # tree tail of the previous chunk opens the DVE stream here,
            # filling what would otherwise be a DVE idle (re-throttle) gap
            if c > 0:
                emit_l23(c - 1)

            # --- chunk-front gates in one PSUM tile: xr = Wir@x + b_r
            #     (cols 0:1024) and the z pre-act (cols 1024:2048); this
            #     12-matmul block is the PE runway that covers ACT's
            #     end-of-previous-chunk lag ---
            pg = pp_pool.tile([128, 2048], F32, tag="pp", name=f"pg_{c}")
            for fi in range(2):
                o = pg[:, fi * 512:(fi + 1) * 512]
                nc.tensor.matmul(o, wcol("wir", 0, fi), xbt[:, 0:512],
                                 start=True, stop=False)
                nc.tensor.matmul(o, wcol("wir", 1, fi), xbt[:, 512:1024],
                                 start=False, stop=True)
            for fi in range(2):
                o = pg[:, 1024 + fi * 512:1024 + (fi + 1) * 512]
                nc.tensor.matmul(o, wcol("wiz", 0, fi), xbt[:, 0:512],
                                 start=True, stop=False)
                nc.tensor.matmul(o, wcol("wiz", 1, fi), xbt[:, 512:1024],
                                 start=False, stop=False)
                nc.tensor.matmul(o, wcol("whz", 0, fi), xbt[:, 1024:1536],
                                 start=False, stop=False)
                nc.tensor.matmul(o, wcol("whz", 1, fi), xbt[:, 1536:2048],
                                 start=False, stop=True)
            xrt = xr_pool.tile([128, 1024], BF16, tag="xr", name=f"xr_{c}")
            for fi in range(2):
                nc.scalar.add(xrt[:, fi * 512:(fi + 1) * 512],
                              pg[:, fi * 512:(fi + 1) * 512],
                              bias_t[:, fi * 3:fi * 3 + 1])
            zt = z_pool.tile([128, 1024], F32, tag="z", name=f"z_{c}")
            for fi in range(2):
                nc.scalar.activation(zt[:, fi * 512:(fi + 1) * 512],
                                     pg[:, 1024 + fi * 512:1024 + (fi + 1) * 512],
                                     SIG, bias=bias_t[:, fi * 3 + 1:fi * 3 + 2])

            rct = rc_pool.tile([128, 4 * 2048], BF16, tag="rc", name=f"rc_{c}")
            pdt = pd_pool.tile([128, 4 * 2048], BF16, tag="pd", name=f"pd_{c}")
            state[c] = {"x": xbt, "hf": hft, "z": zt}
            emit_pair(c, 0, hsc, xrt, rct, pdt)
            emit_pair(c, 1, hsc, xrt, rct, pdt)
            if c == NCH - 1:
                # skewed tree for the final chunk: fold pairs progressively
                # (s = ((p0+p1)+p2)+p3) so the last pair passes through a
                # single add level and the end-of-kernel chain is short
                with nc.allow_low_precision(reason="bf16 neighbor sums"):
                    nc.vector.tensor_add(pdt[:, 0:1024], rct[:, 0:1024],
                                         rct[:, 1024:2048])
            if c > 0:
                emit_mid(c)
            emit_pair(c, 2, hsc, xrt, rct, pdt)
            if c == NCH - 1:
                with nc.allow_low_precision(reason="bf16 neighbor sums"):
                    nc.vector.tensor_add(pdt[:, 1024:2048], pdt[:, 0:1024],
                                         rct[:, 2048:3072])
            emit_pair(c, 3, hsc, xrt, rct, pdt,
                      act_split=(c == NCH - 1))
            if c == NCH - 1:
                sct = s_pool.tile([128, 1024], BF16, tag="s", name=f"s_{c}")
                with nc.allow_low_precision(reason="bf16 neighbor sums"):
                    for fi in range(2):
                        nc.vector.tensor_add(
                            sct[:, fi * 512:(fi + 1) * 512],
                            pdt[:, 1024 + fi * 512:1024 + (fi + 1) * 512],
                            rct[:, 3072 + fi * 512:3072 + (fi + 1) * 512])
                state[c]["s"] = sct
            else:
                pend[c] = (rct, pdt)

        emit_last_tail(NCH - 1)

    nc.compile()
    return nc


def _prep_inputs(x, h_sum, hs, Wir, bir, Whr, bhr, Wiz, biz, Whz, bhz,
                 Win, bin_, Whn, bhn):
    """Shard + pre-chunk to per-core, per-chunk feature-major HBM layouts."""
    f32 = np.float32
    x = np.asarray(x, f32)
    h = np.asarray(h_sum, f32)
    hs = np.asarray(hs, f32)

    # packed weights, need-ordered; wpL[p, W_OFF[w] + k*256 + f*128 + m]
    # = W[f*128+m, k*128+p]; identity at ID_OFF
    wpack = np.zeros((128, WP_COLS), f32)
    for w, W in (("wir", Wir), ("whr", Whr), ("wiz", Wiz), ("whz", Whz),
                 ("win", Win), ("whn", Whn)):
        WT = np.asarray(W, f32).T  # [in, out]
        for k in range(2):
            wpack[:, W_OFF[w] + k * 256:W_OFF[w] + (k + 1) * 256] = \
                WT[k * 128:(k + 1) * 128, :]
    wpack[:, ID_OFF:ID_OFF + 128] = np.eye(128, dtype=f32)
    wpack_bf = np.ascontiguousarray(wpack.astype(BF_NP))

    b_r = np.asarray(bir, f32) + np.asarray(bhr, f32)
    b_z = np.asarray(biz, f32) + np.asarray(bhz, f32)
    b_n = np.asarray(bin_, f32) + np.asarray(bhn, f32)
    biasp = np.empty((128, 6), f32)
    for f in range(2):
        biasp[:, f * 3 + 0] = b_r[f * 128:(f + 1) * 128]
        biasp[:, f * 3 + 1] = b_z[f * 128:(f + 1) * 128]
        biasp[:, f * 3 + 2] = b_n[f * 128:(f + 1) * 128]

    in_maps = []
    for c in range(M):
        sl = slice(c * BL, (c + 1) * BL)
        # x/h: [BL, 256] -> [ch, b, k, p] -> [ch, p, k, b] -> [ch, 128, 1024]
        xc = x[sl].reshape(NCH, CW, 2, 128).transpose(0, 3, 2, 1)
        hc = h[sl].reshape(NCH, CW, 2, 128).transpose(0, 3, 2, 1)
        xb = np.concatenate([xc.astype(BF_NP).reshape(NCH, 128, 1024),
                             hc.astype(BF_NP).reshape(NCH, 128, 1024)], axis=2)
        # hs: [8, BL, 256] -> [pr, j, ch, b, k, p] -> [ch, p, pr, k, j, b]
        hsc = hs[:, sl, :].reshape(NPAIR, 2, NCH, CW, 2, 128)
        m = {
            "xbL": np.ascontiguousarray(xb),
            "hfL": np.ascontiguousarray(hc).reshape(NCH, 128, 1024),
            "hsL": hsc.transpose(2, 5, 0, 4, 1, 3).astype(BF_NP).reshape(
                NCH, 128, 8192),
            "wpL": wpack_bf,
            "biasp": biasp,
        }
        in_maps.append(m)
    return in_maps


def _run(inputs, trace=False, **trace_kwargs):
    global _cached
    if _cached is None:
        _cached = _build()
    nc = _cached
    in_maps = _prep_inputs(**inputs)
    res = run_bass_kernel_spmd(nc, in_maps, list(range(M)), trace=trace,
                               **trace_kwargs)
    out = np.empty((B, H), np.float32)
    for c in range(M):
        # outL [ch, p, (f b)] -> [ch, b, f, p] -> [BL, 256]
        o = res.results[c]["outL"].reshape(NCH, 128, 2, CW)
        out[c * BL:(c + 1) * BL, :] = o.transpose(0, 3, 2, 1).reshape(BL, 256)
    return out, res


def kernel(**inputs):
    return _run(inputs)[0]


# revision 48
# speedup vs baseline: 1.0354x; 1.0121x over previous
"""GRU-style GNN message-passing kernel for Trainium2 (8 NeuronCores, SPMD).

Reference computation (per node b, features 256, 8 neighbors):
    xr = x @ Wir.T + bir
    hr_n = hs_n @ Whr.T + bhr
    r_n = sigmoid(xr + hr_n)
    z = sigmoid(x @ Wiz.T + biz + h_sum @ Whz.T + bhz)
    s = sum_n r_n * hs_n
    n = tanh(x @ Win.T + bin + s @ Whn.T + bhn)
    out = (1 - z) * n + z * h_sum

Strategy: data-parallel over B=32768 across 8 cores (4096 rows each),
8 batch-chunks of 512 per core, feature-major on-chip layout
([256 features = 2 partition chunks of 128, batch free dim]).

The schedule is built around keeping the PE matmul stream dense (any PE
idle gap re-engages the HAM clock throttle and halves the PE clock):

  - Host-side pre-chunked HBM layouts: every DMA is a plain 2D copy
    with 2-16KB contiguous runs; one hs DMA per chunk (split per-pair
    for chunk 0 so pair 0 lands early); x and bf16-h_sum packed in one
    tensor; all weights in one need-ordered pack, DMA'd in two pieces.
  - Per chunk PE stream: a 12-matmul [xr | z-gate] front block (the PE
    runway that covers ACT's end-of-previous-chunk lag), r-matmul
    pairs 0-1, the deferred n-gate of chunk c-1, pairs 2-3.  The DVE
    product tree of chunk c completes at the start of chunk c+1 (which
    also fills DVE's would-be idle/re-throttle gap), so PE never waits
    on it.
  - Each neighbor pair accumulates in one [128,2048] PSUM tile (4
    interleaved 512-wide regions: Whr k0/k1 matmuls + an identity
    matmul that adds the shared xr) and drains with a single wide
    sigmoid ACTIVATE.
  - DVE runs everything alias-free in the packed bf16 2x mode:
    products per pair, the add tree, and the final combine
    out = n + z*(h-n) in fp32 (h_sum kept fp32 for the dominant term).
"""

import sys
import numpy as np
from contextlib import ExitStack

sys.path.insert(0, "/opt/trn_rl_repo")

import ml_dtypes
import concourse.bacc as bacc
import concourse.tile as tile
from concourse import mybir
from concourse.bass_utils import run_bass_kernel_spmd

F32 = mybir.dt.float32
BF16 = mybir.dt.bfloat16
BF_NP = ml_dtypes.bfloat16

N_NEIGH, B, IN, H = 8, 32768, 256, 256
M = 8                    # cores
BL = B // M              # rows per core (4096)
NCH = 8                  # batch chunks per core
CW = BL // NCH           # chunk width (512)
NPAIR = N_NEIGH // 2     # neighbor pairs (4)

_cached = None  # compiled program, reused across kernel() calls

SIG = mybir.ActivationFunctionType.Sigmoid
TANH = mybir.ActivationFunctionType.Tanh

# weight pack column offsets (need-ordered: xr gate, z gate, r pairs, n)
W_OFF = {"wir": 0, "wiz": 512, "whz": 1024, "whr": 1536, "win": 2176,
         "whn": 2688}
ID_OFF = 2048
WP_COLS = 3200
WP_SPLIT = 1536  # piece A: wir/wiz/whz; piece B: whr/id/win/whn


def _build():
    nc = bacc.Bacc("TRN2", target_bir_lowering=False, debug=False, num_devices=M)

    # xbL packs x (cols 0:1024) and h_sum-bf16 (cols 1024:2048) per chunk
    xbL = nc.dram_tensor("xbL", [NCH, 128, 2048], BF16, kind="ExternalInput").ap()
    hfL = nc.dram_tensor("hfL", [NCH, 128, 1024], F32, kind="ExternalInput").ap()
    hsL = nc.dram_tensor("hsL", [NCH, 128, 8192], BF16,
                         kind="ExternalInput").ap()
    wpL = nc.dram_tensor("wpL", [128, WP_COLS], BF16, kind="ExternalInput").ap()
    # bias pack: col f*3+j holds feature-chunk f of (b_r, b_z, b_n)[j]
    biasp = nc.dram_tensor("biasp", [128, 6], F32, kind="ExternalInput").ap()
    outL = nc.dram_tensor("outL", [NCH, 128, 1024], F32, kind="ExternalOutput").ap()

    with tile.TileContext(nc) as tc, ExitStack() as ctx:
        cpool = ctx.enter_context(tc.tile_pool(name="const", bufs=1))
        x_pool = ctx.enter_context(tc.tile_pool(name="x", bufs=3))
        hf_pool = ctx.enter_context(tc.tile_pool(name="hf", bufs=3))
        hs_pool = ctx.enter_context(tc.tile_pool(name="hs", bufs=3))
        xr_pool = ctx.enter_context(tc.tile_pool(name="xr", bufs=2))
        z_pool = ctx.enter_context(tc.tile_pool(name="z", bufs=2))
        rc_pool = ctx.enter_context(tc.tile_pool(name="rc", bufs=2))
        pd_pool = ctx.enter_context(tc.tile_pool(name="pd", bufs=2))
        s_pool = ctx.enter_context(tc.tile_pool(name="s", bufs=2))
        n_pool = ctx.enter_context(tc.tile_pool(name="n", bufs=2))
        d_pool = ctx.enter_context(tc.tile_pool(name="d", bufs=2))
        o_pool = ctx.enter_context(tc.tile_pool(name="o", bufs=2))
        pp_pool = ctx.enter_context(tc.tile_pool(name="pp", bufs=2, space="PSUM"))

        # --- constants: weight pack in two need-ordered DMAs + biases ---
        # wir (the first matmuls' stationary) rides alone so it lands first;
        # wiz/whz follow once chunk 0's x data is queued
        wp_t = cpool.tile([128, WP_COLS], BF16, tag="wp", name="wp_t")
        nc.sync.dma_start(out=wp_t[:, 0:512], in_=wpL[:, 0:512])
        bias_t = cpool.tile([128, 6], F32, tag="biasp", name="bias_t")
        nc.sync.dma_start(out=bias_t[:, :], in_=biasp[:, :])
        # piece B (whr/id/win/whn) is DMA'd inside chunk 0, after the data
        # the very first matmuls need, so PE starts ~6us earlier

        # warm-up: the PE HAM clock-gate needs ~3.4us of sustained activity
        # to lift the 1.2GHz cold throttle. Run dummy matmuls on a zeroed
        # tile during the startup DMA window so real work starts at 2.4GHz.
        wu_t = cpool.tile([128, 128], BF16, tag="wu", name="wu_t")
        nc.vector.memset(wu_t[:, :], 0)
        pwu = pp_pool.tile([128, 2048], F32, tag="pp", name="pwu")
        for i in range(44):
            nc.tensor.matmul(pwu[:, (i % 4) * 512:(i % 4) * 512 + 128],
                             wu_t[:, :], wu_t[:, :], start=True, stop=True)

        def wcol(w, k, f):  # stationary [128,128] for weight w, k-chunk, f-chunk
            off = W_OFF[w] + k * 256 + f * 128
            return wp_t[:, off:off + 128]

        id_t = wp_t[:, ID_OFF:ID_OFF + 128]

        state = {}  # chunk -> tiles needed by the deferred n-gate/combine

        def emit_mid(c):
            """Between pairs 1 and 2 of chunk c: the deferred n-gate of
            chunk c-1, then its combine on DVE + store."""
            st = state.pop(c - 1)
            pn = pp_pool.tile([128, 2048], F32, tag="pp", name=f"pn_{c - 1}")
            for fi in range(2):
                o = pn[:, fi * 512:(fi + 1) * 512]
                nc.tensor.matmul(o, wcol("win", 0, fi), st["x"][:, 0:512],
                                 start=True, stop=False)
                nc.tensor.matmul(o, wcol("win", 1, fi), st["x"][:, 512:1024],
                                 start=False, stop=False)
                nc.tensor.matmul(o, wcol("whn", 0, fi), st["s"][:, 0:512],
                                 start=False, stop=False)
                nc.tensor.matmul(o, wcol("whn", 1, fi), st["s"][:, 512:1024],
                                 start=False, stop=True)
            nt = n_pool.tile([128, 1024], F32, tag="n", name=f"n_{c - 1}")
            for fi in range(2):
                nc.scalar.activation(nt[:, fi * 512:(fi + 1) * 512],
                                     pn[:, fi * 512:(fi + 1) * 512], TANH,
                                     bias=bias_t[:, fi * 3 + 2:fi * 3 + 3])
            # out = n + z * (h - n) on DVE, then store
            dt_ = d_pool.tile([128, 1024], F32, tag="d", name=f"d_{c - 1}")
            nc.vector.tensor_sub(dt_[:, :], st["hf"][:, :], nt[:, :])
            nc.vector.tensor_mul(dt_[:, :], st["z"][:, :], dt_[:, :])
            ot = o_pool.tile([128, 1024], F32, tag="o", name=f"o_{c - 1}")
            nc.vector.tensor_add(ot[:, :], nt[:, :], dt_[:, :])
            nc.sync.dma_start(out=outL[c - 1], in_=ot[:, :])

        def emit_last_tail(cc):
            """n-gate + combine for the final chunk, f-split to shorten the
            end-of-kernel serial chain."""
            st = state.pop(cc)
            pn = pp_pool.tile([128, 2048], F32, tag="pp", name=f"pn_{cc}")
            nt = n_pool.tile([128, 1024], F32, tag="n", name=f"n_{cc}")
            dt_ = d_pool.tile([128, 1024], F32, tag="d", name=f"d_{cc}")
            ot = o_pool.tile([128, 1024], F32, tag="o", name=f"o_{cc}")
            for fi in range(2):
                o = pn[:, fi * 512:(fi + 1) * 512]
                nc.tensor.matmul(o, wcol("win", 0, fi), st["x"][:, 0:512],
                                 start=True, stop=False)
                nc.tensor.matmul(o, wcol("win", 1, fi), st["x"][:, 512:1024],
                                 start=False, stop=False)
                nc.tensor.matmul(o, wcol("whn", 0, fi), st["s"][:, 0:512],
                                 start=False, stop=False)
                nc.tensor.matmul(o, wcol("whn", 1, fi), st["s"][:, 512:1024],
                                 start=False, stop=True)
                nc.scalar.activation(nt[:, fi * 512:(fi + 1) * 512],
                                     pn[:, fi * 512:(fi + 1) * 512], TANH,
                                     bias=bias_t[:, fi * 3 + 2:fi * 3 + 3])
                s_ = slice(fi * 512, (fi + 1) * 512)
                nc.vector.tensor_sub(dt_[:, s_], st["hf"][:, s_], nt[:, s_])
                nc.vector.tensor_mul(dt_[:, s_], st["z"][:, s_], dt_[:, s_])
                nc.vector.tensor_add(ot[:, s_], nt[:, s_], dt_[:, s_])
                nc.sync.dma_start(out=outL[cc][:, s_], in_=ot[:, s_])

        def emit_pair(c, p, hsc, xrt, rct, pdt, act_split=False):
            base = p * 2048
            # one PSUM tile per pair, (f, j, b) layout; four interleaved
            # 512-wide accumulation regions. whr matmuls first, the xr
            # identity adds last so xr is never waited on.
            pra = pp_pool.tile([128, 2048], F32, tag="pp", name=f"pr{p}_{c}")
            for fi in range(2):
                oj0 = pra[:, fi * 1024:fi * 1024 + 512]
                oj1 = pra[:, fi * 1024 + 512:fi * 1024 + 1024]
                nc.tensor.matmul(oj0, wcol("whr", 0, fi),
                                 hsc[:, base:base + 512],
                                 start=True, stop=False)
                nc.tensor.matmul(oj1, wcol("whr", 0, fi),
                                 hsc[:, base + 512:base + 1024],
                                 start=True, stop=False)
                nc.tensor.matmul(oj0, wcol("whr", 1, fi),
                                 hsc[:, base + 1024:base + 1536],
                                 start=False, stop=False)
                nc.tensor.matmul(oj1, wcol("whr", 1, fi),
                                 hsc[:, base + 1536:base + 2048],
                                 start=False, stop=False)
            for fi in range(2):
                nc.tensor.matmul(pra[:, fi * 1024:fi * 1024 + 512], id_t,
                                 xrt[:, fi * 512:(fi + 1) * 512],
                                 start=False, stop=True)
                nc.tensor.matmul(pra[:, fi * 1024 + 512:fi * 1024 + 1024],
                                 id_t, xrt[:, fi * 512:(fi + 1) * 512],
                                 start=False, stop=True)
            if act_split:
                # f-split activation/product path (used for the final pair
                # of the final chunk to shorten the tail chain)
                for fi in range(2):
                    hb = base + fi * 1024
                    nc.scalar.activation(rct[:, hb:hb + 1024],
                                         pra[:, fi * 1024:(fi + 1) * 1024], SIG)
                    nc.vector.tensor_mul(pdt[:, hb:hb + 1024],
                                         rct[:, hb:hb + 1024],
                                         hsc[:, hb:hb + 1024])
                    with nc.allow_low_precision(reason="bf16 neighbor sums"):
                        nc.vector.tensor_add(
                            rct[:, p * 1024 + fi * 512:p * 1024 + fi * 512 + 512],
                            pdt[:, hb:hb + 512],
                            pdt[:, hb + 512:hb + 1024])
            else:
                # r for pair p, both f chunks in one activation
                nc.scalar.activation(rct[:, base:base + 2048], pra[:, :], SIG)
                # products r*hs for the whole pair block (alias-free so the
                # DVE packed bf16 mode stays eligible)
                blk = slice(base, base + 2048)
                nc.vector.tensor_mul(pdt[:, blk], rct[:, blk], hsc[:, blk])
                # tree level 1: j0 + j1 per f chunk -> rc cols [p*1024, +1024)
                with nc.allow_low_precision(reason="bf16 neighbor sums"):
                    for fi in range(2):
                        nc.vector.tensor_add(
                            rct[:, p * 1024 + fi * 512:p * 1024 + fi * 512 + 512],
                            pdt[:, base + fi * 1024:base + fi * 1024 + 512],
                            pdt[:, base + fi * 1024 + 512:base + fi * 1024 + 1024])

        pend = {}  # chunk -> (rct, pdt) awaiting tree levels 2+3

        def emit_l23(cc, skip_l2a=False):
            rct, pdt = pend.pop(cc)
            sct = s_pool.tile([128, 1024], BF16, tag="s", name=f"s_{cc}")
            with nc.allow_low_precision(reason="bf16 neighbor sums"):
                if not skip_l2a:
                    nc.vector.tensor_add(pdt[:, 0:1024], rct[:, 0:1024],
                                         rct[:, 1024:2048])
                nc.vector.tensor_add(pdt[:, 1024:2048], rct[:, 2048:3072],
                                     rct[:, 3072:4096])
                nc.vector.tensor_add(sct[:, :], pdt[:, 0:1024],
                                     pdt[:, 1024:2048])
            state[cc]["s"] = sct

        for c in range(NCH):
            # --- input DMAs (plain 2D copies, 4-16KB contiguous runs);
            #     chunk 0's hs comes in per-pair so pair0 lands early ---
            xbt = x_pool.tile([128, 2048], BF16, tag="x", name=f"x_{c}")
            hsc = hs_pool.tile([128, 8192], BF16, tag="hs", name=f"hs_{c}")
            if c == 0:
                # critical-path order: x half -> wiz/whz -> h half -> pair0
                # -> piece B of the weights -> the rest
                nc.sync.dma_start(out=xbt[:, 0:1024], in_=xbL[c][:, 0:1024])
                nc.sync.dma_start(out=wp_t[:, 512:WP_SPLIT],
                                  in_=wpL[:, 512:WP_SPLIT])
                nc.sync.dma_start(out=xbt[:, 1024:2048],
                                  in_=xbL[c][:, 1024:2048])
                nc.sync.dma_start(out=hsc[:, 0:2048], in_=hsL[c][:, 0:2048])
                nc.sync.dma_start(out=wp_t[:, WP_SPLIT:WP_COLS],
                                  in_=wpL[:, WP_SPLIT:WP_COLS])
                for p in range(1, NPAIR):
                    nc.sync.dma_start(out=hsc[:, p * 2048:(p + 1) * 2048],
                                      in_=hsL[c][:, p * 2048:(p + 1) * 2048])
            else:
                nc.sync.dma_start(out=xbt[:, :], in_=xbL[c])
                nc.sync.dma_start(out=hsc[:, :], in_=hsL[c])
            hft = hf_pool.tile([128, 1024], F32, tag="hf", name=f"hf_{c}")
            nc.sync.dma_start(out=hft[:, :], in_=hfL[c])

            # tree tail of the previous chunk opens the DVE stream here,
            # filling what would otherwise be a DVE idle (re-throttle) gap
            if c > 0:
                emit_l23(c - 1)

            # --- chunk-front gates in one PSUM tile: xr = Wir@x + b_r
            #     (cols 0:1024) and the z pre-act (cols 1024:2048); this
            #     12-matmul block is the PE runway that covers ACT's
            #     end-of-previous-chunk lag ---
            pg = pp_pool.tile([128, 2048], F32, tag="pp", name=f"pg_{c}")
            for fi in range(2):
                o = pg[:, fi * 512:(fi + 1) * 512]
                nc.tensor.matmul(o, wcol("wir", 0, fi), xbt[:, 0:512],
                                 start=True, stop=False)
                nc.tensor.matmul(o, wcol("wir", 1, fi), xbt[:, 512:1024],
                                 start=False, stop=True)
            for fi in range(2):
                o = pg[:, 1024 + fi * 512:1024 + (fi + 1) * 512]
                nc.tensor.matmul(o, wcol("wiz", 0, fi), xbt[:, 0:512],
                                 start=True, stop=False)
                nc.tensor.matmul(o, wcol("wiz", 1, fi), xbt[:, 512:1024],
                                 start=False, stop=False)
                nc.tensor.matmul(o, wcol("whz", 0, fi), xbt[:, 1024:1536],
                                 start=False, stop=False)
                nc.tensor.matmul(o, wcol("whz", 1, fi), xbt[:, 1536:2048],
                                 start=False, stop=True)
            xrt = xr_pool.tile([128, 1024], BF16, tag="xr", name=f"xr_{c}")
            for fi in range(2):
                nc.scalar.add(xrt[:, fi * 512:(fi + 1) * 512],
                              pg[:, fi * 512:(fi + 1) * 512],
                              bias_t[:, fi * 3:fi * 3 + 1])
            zt = z_pool.tile([128, 1024], F32, tag="z", name=f"z_{c}")
            for fi in range(2):
                nc.scalar.activation(zt[:, fi * 512:(fi + 1) * 512],
                                     pg[:, 1024 + fi * 512:1024 + (fi + 1) * 512],
                                     SIG, bias=bias_t[:, fi * 3 + 1:fi * 3 + 2])

            rct = rc_pool.tile([128, 4 * 2048], BF16, tag="rc", name=f"rc_{c}")
            pdt = pd_pool.tile([128, 4 * 2048], BF16, tag="pd", name=f"pd_{c}")
            state[c] = {"x": xbt, "hf": hft, "z": zt}
            emit_pair(c, 0, hsc, xrt, rct, pdt)
            emit_pair(c, 1, hsc, xrt, rct, pdt)
            if c == NCH - 1:
                # skewed tree for the final chunk: fold pairs progressively
                # (s = ((p0+p1)+p2)+p3) so the last pair passes through a
                # single add level and the end-of-kernel chain is short
                with nc.allow_low_precision(reason="bf16 neighbor sums"):
                    nc.vector.tensor_add(pdt[:, 0:1024], rct[:, 0:1024],
                                         rct[:, 1024:2048])
            if c > 0:
                emit_mid(c)
            emit_pair(c, 2, hsc, xrt, rct, pdt)
            if c == NCH - 1:
                with nc.allow_low_precision(reason="bf16 neighbor sums"):
                    nc.vector.tensor_add(pdt[:, 1024:2048], pdt[:, 0:1024],
                                         rct[:, 2048:3072])
            emit_pair(c, 3, hsc, xrt, rct, pdt,
                      act_split=(c == NCH - 1))
            if c == NCH - 1:
                sct = s_pool.tile([128, 1024], BF16, tag="s", name=f"s_{c}")
                with nc.allow_low_precision(reason="bf16 neighbor sums"):
                    for fi in range(2):
                        nc.vector.tensor_add(
                            sct[:, fi * 512:(fi + 1) * 512],
                            pdt[:, 1024 + fi * 512:1024 + (fi + 1) * 512],
                            rct[:, 3072 + fi * 512:3072 + (fi + 1) * 512])
                state[c]["s"] = sct
            else:
                pend[c] = (rct, pdt)

        emit_last_tail(NCH - 1)

    nc.compile()
    return nc


def _prep_inputs(x, h_sum, hs, Wir, bir, Whr, bhr, Wiz, biz, Whz, bhz,
                 Win, bin_, Whn, bhn):
    """Shard + pre-chunk to per-core, per-chunk feature-major HBM layouts."""
    f32 = np.float32
    x = np.asarray(x, f32)
    h = np.asarray(h_sum, f32)
    hs = np.asarray(hs, f32)

    # packed weights, need-ordered; wpL[p, W_OFF[w] + k*256 + f*128 + m]
    # = W[f*128+m, k*128+p]; identity at ID_OFF
    wpack = np.zeros((128, WP_COLS), f32)
    for w, W in (("wir", Wir), ("whr", Whr), ("wiz", Wiz), ("whz", Whz),
                 ("win", Win), ("whn", Whn)):
        WT = np.asarray(W, f32).T  # [in, out]
        for k in range(2):
            wpack[:, W_OFF[w] + k * 256:W_OFF[w] + (k + 1) * 256] = \
                WT[k * 128:(k + 1) * 128, :]
    wpack[:, ID_OFF:ID_OFF + 128] = np.eye(128, dtype=f32)
    wpack_bf = np.ascontiguousarray(wpack.astype(BF_NP))

    b_r = np.asarray(bir, f32) + np.asarray(bhr, f32)
    b_z = np.asarray(biz, f32) + np.asarray(bhz, f32)
    b_n = np.asarray(bin_, f32) + np.asarray(bhn, f32)
    biasp = np.empty((128, 6), f32)
    for f in range(2):
        biasp[:, f * 3 + 0] = b_r[f * 128:(f + 1) * 128]
        biasp[:, f * 3 + 1] = b_z[f * 128:(f + 1) * 128]
        biasp[:, f * 3 + 2] = b_n[f * 128:(f + 1) * 128]

    in_maps = []
    for c in range(M):
        sl = slice(c * BL, (c + 1) * BL)
        # x/h: [BL, 256] -> [ch, b, k, p] -> [ch, p, k, b] -> [ch, 128, 1024]
        xc = x[sl].reshape(NCH, CW, 2, 128).transpose(0, 3, 2, 1)
        hc = h[sl].reshape(NCH, CW, 2, 128).transpose(0, 3, 2, 1)
        xb = np.concatenate([xc.astype(BF_NP).reshape(NCH, 128, 1024),
                             hc.astype(BF_NP).reshape(NCH, 128, 1024)], axis=2)
        # hs: [8, BL, 256] -> [pr, j, ch, b, k, p] -> [ch, p, pr, k, j, b]
        hsc = hs[:, sl, :].reshape(NPAIR, 2, NCH, CW, 2, 128)
        m = {
            "xbL": np.ascontiguousarray(xb),
            "hfL": np.ascontiguousarray(hc).reshape(NCH, 128, 1024),
            "hsL": hsc.transpose(2, 5, 0, 4, 1, 3).astype(BF_NP).reshape(
                NCH, 128, 8192),
            "wpL": wpack_bf,
            "biasp": biasp,
        }
        in_maps.append(m)
    return in_maps


def _run(inputs, trace=False, **trace_kwargs):
    global _cached
    if _cached is None:
        _cached = _build()
    nc = _cached
    in_maps = _prep_inputs(**inputs)
    res = run_bass_kernel_spmd(nc, in_maps, list(range(M)), trace=trace,
                               **trace_kwargs)
    out = np.empty((B, H), np.float32)
    for c in range(M):
        # outL [ch, p, (f b)] -> [ch, b, f, p] -> [BL, 256]
        o = res.results[c]["outL"].reshape(NCH, 128, 2, CW)
        out[c * BL:(c + 1) * BL, :] = o.transpose(0, 3, 2, 1).reshape(BL, 256)
    return out, res


def kernel(**inputs):
    return _run(inputs)[0]
